# revision 8
# baseline (speedup 1.0000x reference)
"""ObjectDecoder kernel for Trainium2 (8 NeuronCores, data-parallel over batch).

Computes out[b, o, a, p, k] = sum_d x[b, o, d] * W[o, a, p, d, k] + bias[o, a, p, k]
  x: [16384, 16, 256] f32, W: [16, 4, 2, 256, 8] f32, b: [16, 4, 2, 8] f32
  out: [16384, 16, 4, 2, 8] f32

Per-core plan (batch shard of 2048 rows):
  - The kernel is HBM-bandwidth-bound (43 MB/core in fp32), so x, W and the
    output travel as bf16 (host casts; fp32 PSUM accumulate keeps the
    contraction exact enough: ~1e-3 max rel err vs the 2e-2 gate). Bias is
    added in fp32 on the scalar engine before the output rounds to bf16.
  - x shard is pre-transposed on host to xt[obj, d_lo, d_hi, batch] so the
    contraction dim (d) lands on SBUF partitions and every DMA is a large
    contiguous block (8 KiB per partition line).
  - W is pre-arranged to wt[d_lo(128), k_chunk(2), obj(16), apk(64)]; bias to
    bt[(pair_half*64+apk)(128), pair(8)] (fp32).
  - For each pair of objects: 4 matmuls [K=128, M=64, N=512] accumulate into a
    [128, 512] PSUM bank (objects 2i / 2i+1 stacked on partitions); the scalar
    engine evacuates PSUM with a fused per-partition bias add, rounding to
    bf16; result stores to out_t[obj, apk, batch] in DRAM, un-transposed and
    upcast on host at the end.
"""

import os
from contextlib import ExitStack

os.environ.setdefault("JAX_PLATFORMS", "axon")

import numpy as np
import ml_dtypes

import concourse.bass as bass
import concourse.mybir as mybir
import concourse.tile as tile
from concourse import bacc
from concourse.bass_utils import run_bass_kernel_spmd

B, N_OBJ, DIM_IN, APK = 16384, 16, 256, 64
N_CORES = 8
BS = B // N_CORES          # 2048 batch rows per core
NT = 512                   # moving-operand tile (one PSUM bank of fp32)
NB = BS // NT              # 4 batch chunks per core
F32 = mybir.dt.float32
BF16 = mybir.dt.bfloat16
NP_BF16 = ml_dtypes.bfloat16

_CACHE: dict = {}


def _build_nc(variant=None):
    if variant is None:
        variant = os.environ.get("KVARIANT", "v7")
    nc = bacc.Bacc("TRN2", target_bir_lowering=False, debug=False)

    # xt[o, p, k, b]: d = k*128 + p — 8KiB contiguous per partition line
    xt = nc.declare_dram_parameter("xt", [N_OBJ, 128, 2, BS], BF16, isOutput=False)
    wt = nc.declare_dram_parameter("wt", [128, 2, N_OBJ, APK], BF16, isOutput=False)
    bt = nc.declare_dram_parameter("bt", [128, N_OBJ // 2], F32, isOutput=False)
    out = nc.declare_dram_parameter("out", [N_OBJ, APK, BS], BF16, isOutput=True)

    with tile.TileContext(nc) as tc, ExitStack() as ctx:
        wpool = ctx.enter_context(tc.tile_pool(name="w", bufs=1))
        n_fine = 1
        xpool = ctx.enter_context(tc.tile_pool(name="x", bufs=10))
        fpool = ctx.enter_context(tc.tile_pool(name="xf", bufs=2 * n_fine))
        psum = ctx.enter_context(
            tc.tile_pool(name="ps", bufs=8, space=bass.MemorySpace.PSUM)
        )
        opool = ctx.enter_context(tc.tile_pool(name="o", bufs=3))

        # W/bias via the scalar HWDGE queue: it is idle until the first
        # activation (~20us in), and HWDGE starts ~4us earlier than SWDGE —
        # the first matmul is gated on W's arrival.
        w_sb = wpool.tile([128, 2, N_OBJ, APK], BF16)
        nc.scalar.dma_start(w_sb[:], wt[:])
        b_sb = wpool.tile([128, N_OBJ // 2], F32)
        nc.scalar.dma_start(b_sb[:], bt[:])

        n_pairs = N_OBJ // 2
        for op in range(n_pairs):  # object pairs
            # Last pair: finer loads/stores to shrink the pipeline-drain
            # tail (nothing left to overlap the final compute+stores with).
            fine = op >= n_pairs - n_fine
            xts = {}
            for o2 in range(2):
                pool = fpool if fine else xpool
                t = pool.tile([128, 2, BS], BF16)
                if fine:
                    # batch-half loads (4KiB lines); order o0h0,o1h0,o0h1,o1h1
                    # handled below by issuing h loops outer over objects
                    pass
                else:
                    nc.sync.dma_start(t[:], xt[2 * op + o2])
                for k in range(2):
                    xts[o2, k] = t[:, k, :]
                xts[o2, "t"] = t
            if fine:
                # both objects' first batch-half before either second half, so
                # the first half's chunks can compute while h1 still loads
                for h in range(2):
                    hs = h * (BS // 2)
                    for o2 in range(2):
                        nc.sync.dma_start(
                            xts[o2, "t"][:, :, hs : hs + BS // 2],
                            xt[2 * op + o2, :, :, hs : hs + BS // 2],
                        )
            ot = opool.tile([128, BS], BF16)
            # NT=512: matmul moving free dim is capped by the fp32 PSUM bank
            nt = NT
            nb = BS // nt
            for n in range(nb):
                ps = psum.tile([128, nt], F32)
                # o2 innermost: consecutive matmuls target PE column strips
                # 0/64 alternately, so LDWEIGHTS(i+1) overlaps MATMUL(i)
                for k in range(2):
                    for o2 in range(2):
                        nc.tensor.matmul(
                            ps[o2 * 64 : (o2 + 1) * 64, :],
                            w_sb[:, k, 2 * op + o2, :],
                            xts[o2, k][:, n * nt : (n + 1) * nt],
                            start=(k == 0),
                            stop=(k == 1),
                        )
                nc.scalar.activation(
                    ot[:, n * nt : (n + 1) * nt],
                    ps[:],
                    mybir.ActivationFunctionType.Identity,
                    bias=b_sb[:, op : op + 1],
                )
                if fine:
                    # fine stores stay on the scalar engine: issuing from the
                    # same engine as the ACT guarantees the PSUM-evacuation
                    # writes are visible before the DMA reads them (a sync-
                    # queue store was observed to race the ACT rarely)
                    nc.scalar.dma_start(
                        out[2 * op : 2 * op + 2, :, n * nt : (n + 1) * nt],
                        ot[:, n * nt : (n + 1) * nt],
                    )
                elif op == n_pairs - n_fine - 1:
                    # second-to-last pair: store per chunk (4KiB lines)
                    # so its store doesn't wait on the whole pair's compute
                    nc.scalar.dma_start(
                        out[2 * op : 2 * op + 2, :, n * nt : (n + 1) * nt],
                        ot[:, n * nt : (n + 1) * nt],
                    )
            if not fine and op != n_pairs - n_fine - 1:
                nc.scalar.dma_start(out[2 * op : 2 * op + 2, :, :], ot[:])

    nc.compile()
    return nc


def _get_nc():
    if "nc" not in _CACHE:
        _CACHE["nc"] = _build_nc()
    return _CACHE["nc"]


def _prep_inputs(x, W, b):
    x = np.asarray(x, dtype=np.float32).astype(NP_BF16)
    # wt[d_lo, k_chunk, o, apk]: W[o,a,p,d,k] -> [d,o,apk] -> [2,128,o,apk] -> [128,2,o,apk]
    wt = np.ascontiguousarray(
        np.asarray(W, dtype=np.float32)
        .astype(NP_BF16)
        .transpose(3, 0, 1, 2, 4)
        .reshape(2, 128, N_OBJ, APK)
        .transpose(1, 0, 2, 3)
    )
    # bt[o2*64+apk, pair] — fp32
    bt = np.ascontiguousarray(
        np.asarray(b, dtype=np.float32)
        .reshape(N_OBJ // 2, 2, APK)
        .transpose(1, 2, 0)
        .reshape(128, N_OBJ // 2)
    )
    in_maps = []
    for c in range(N_CORES):
        xs = x[c * BS : (c + 1) * BS]  # [BS, 16, 256] bf16
        # xt[o, p, k, b] with d = k*128 + p (8KiB contiguous per (o, p))
        xt = np.ascontiguousarray(
            xs.transpose(1, 2, 0).reshape(N_OBJ, 2, 128, BS).transpose(0, 2, 1, 3)
        )
        in_maps.append({"xt": xt, "wt": wt, "bt": bt})
    return in_maps


def kernel(x, W, b, _trace=False, **run_kwargs):
    nc = _get_nc()
    in_maps = _prep_inputs(x, W, b)
    res = run_bass_kernel_spmd(
        nc, in_maps, core_ids=list(range(N_CORES)), trace=_trace, **run_kwargs
    )
    _CACHE["last_results"] = res
    out = np.empty((B, N_OBJ, APK), dtype=np.float32)
    for c in range(N_CORES):
        # out_t[o, apk, batch] -> [batch, o, apk]
        out[c * BS : (c + 1) * BS] = (
            res.results[c]["out"].astype(np.float32).transpose(2, 0, 1)
        )
    return out.reshape(B, N_OBJ, 4, 2, 8)


# revision 9
# speedup vs baseline: 1.1159x; 1.1159x over previous
"""ObjectDecoder kernel for Trainium2 (8 NeuronCores, data-parallel over batch).

Computes out[b, o, a, p, k] = sum_d x[b, o, d] * W[o, a, p, d, k] + bias[o, a, p, k]
  x: [16384, 16, 256] f32, W: [16, 4, 2, 256, 8] f32, b: [16, 4, 2, 8] f32
  out: [16384, 16, 4, 2, 8] f32

Per-core plan (batch shard of 2048 rows):
  - The kernel is HBM-bandwidth-bound (43 MB/core in fp32), so x, W and the
    output travel as bf16 (host casts; fp32 PSUM accumulate keeps the
    contraction exact enough: ~1e-3 max rel err vs the 2e-2 gate). Bias is
    added in fp32 on the scalar engine before the output rounds to bf16.
  - x shard is pre-transposed on host to xt[obj, d_lo, d_hi, batch] so the
    contraction dim (d) lands on SBUF partitions and every DMA is a large
    contiguous block (8 KiB per partition line).
  - W is pre-arranged to wt[d_lo(128), k_chunk(2), obj(16), apk(64)]; bias to
    bt[(pair_half*64+apk)(128), pair(8)] (fp32).
  - For each pair of objects: 4 matmuls [K=128, M=64, N=512] accumulate into a
    [128, 512] PSUM bank (objects 2i / 2i+1 stacked on partitions); the scalar
    engine evacuates PSUM with a fused per-partition bias add, rounding to
    bf16; result stores to out_t[obj, apk, batch] in DRAM, un-transposed and
    upcast on host at the end.
"""

import os
from contextlib import ExitStack

os.environ.setdefault("JAX_PLATFORMS", "axon")

import numpy as np
import ml_dtypes

import concourse.bass as bass
import concourse.mybir as mybir
import concourse.tile as tile
from concourse import bacc
from concourse.bass_utils import run_bass_kernel_spmd

B, N_OBJ, DIM_IN, APK = 16384, 16, 256, 64
N_CORES = 8
BS = B // N_CORES          # 2048 batch rows per core
NT = 512                   # moving-operand tile (one PSUM bank of fp32)
NB = BS // NT              # 4 batch chunks per core
F32 = mybir.dt.float32
BF16 = mybir.dt.bfloat16
NP_BF16 = ml_dtypes.bfloat16

_CACHE: dict = {}


def _build_nc(variant=None):
    if variant is None:
        variant = os.environ.get("KVARIANT", "v7")
    nc = bacc.Bacc("TRN2", target_bir_lowering=False, debug=False)

    # xt[o, p, k, b]: d = k*128 + p — 8KiB contiguous per partition line
    xt = nc.declare_dram_parameter("xt", [N_OBJ, 128, 2, BS], BF16, isOutput=False)
    wt = nc.declare_dram_parameter("wt", [128, 2, N_OBJ, APK], BF16, isOutput=False)
    bt = nc.declare_dram_parameter("bt", [128, N_OBJ // 2], F32, isOutput=False)
    out = nc.declare_dram_parameter("out", [N_OBJ, APK, BS], BF16, isOutput=True)

    with tile.TileContext(nc) as tc, ExitStack() as ctx:
        wpool = ctx.enter_context(tc.tile_pool(name="w", bufs=1))
        n_fine = 1
        xpool = ctx.enter_context(tc.tile_pool(name="x", bufs=10))
        fpool = ctx.enter_context(tc.tile_pool(name="xf", bufs=2 * n_fine))
        psum = ctx.enter_context(
            tc.tile_pool(name="ps", bufs=8, space=bass.MemorySpace.PSUM)
        )
        opool = ctx.enter_context(tc.tile_pool(name="o", bufs=3))

        # W/bias via the scalar HWDGE queue: it is idle until the first
        # activation (~20us in), and HWDGE starts ~4us earlier than SWDGE —
        # the first matmul is gated on W's arrival.
        w_sb = wpool.tile([128, 2, N_OBJ, APK], BF16)
        nc.scalar.dma_start(w_sb[:], wt[:])
        b_sb = wpool.tile([128, N_OBJ // 2], F32)
        nc.scalar.dma_start(b_sb[:], bt[:])

        n_pairs = N_OBJ // 2
        for op in range(n_pairs):  # object pairs
            # Last pair: finer loads/stores to shrink the pipeline-drain
            # tail (nothing left to overlap the final compute+stores with).
            fine = op >= n_pairs - n_fine
            xts = {}
            for o2 in range(2):
                pool = fpool if fine else xpool
                t = pool.tile([128, 2, BS], BF16)
                if fine:
                    # batch-half loads (4KiB lines); order o0h0,o1h0,o0h1,o1h1
                    # handled below by issuing h loops outer over objects
                    pass
                else:
                    nc.sync.dma_start(t[:], xt[2 * op + o2])
                for k in range(2):
                    xts[o2, k] = t[:, k, :]
                xts[o2, "t"] = t
            if fine:
                # both objects' first batch-half before either second half, so
                # the first half's chunks can compute while h1 still loads
                for h in range(2):
                    hs = h * (BS // 2)
                    for o2 in range(2):
                        nc.sync.dma_start(
                            xts[o2, "t"][:, :, hs : hs + BS // 2],
                            xt[2 * op + o2, :, :, hs : hs + BS // 2],
                        )
            ot = opool.tile([128, BS], BF16)
            # NT=512: matmul moving free dim is capped by the fp32 PSUM bank
            nt = NT
            nb = BS // nt
            for n in range(nb):
                ps = psum.tile([128, nt], F32)
                # o2 innermost: consecutive matmuls target PE column strips
                # 0/64 alternately, so LDWEIGHTS(i+1) overlaps MATMUL(i)
                for k in range(2):
                    for o2 in range(2):
                        nc.tensor.matmul(
                            ps[o2 * 64 : (o2 + 1) * 64, :],
                            w_sb[:, k, 2 * op + o2, :],
                            xts[o2, k][:, n * nt : (n + 1) * nt],
                            start=(k == 0),
                            stop=(k == 1),
                        )
                nc.scalar.activation(
                    ot[:, n * nt : (n + 1) * nt],
                    ps[:],
                    mybir.ActivationFunctionType.Identity,
                    bias=b_sb[:, op : op + 1],
                )
                # fine stores stay on the scalar engine: issuing from the
                # same engine as the ACT guarantees the PSUM-evacuation
                # writes are visible before the DMA reads them (a sync-
                # queue store was observed to race the ACT rarely).
                # Store per batch-half (not per chunk): fewer 600ns DMA-issue
                # ops in scalar's strict-FIFO queue during the pipeline drain.
                if fine and n % 2 == 1:
                    hs = (n - 1) * nt
                    nc.scalar.dma_start(
                        out[2 * op : 2 * op + 2, :, hs : hs + 2 * nt],
                        ot[:, hs : hs + 2 * nt],
                    )
            if not fine:
                nc.scalar.dma_start(out[2 * op : 2 * op + 2, :, :], ot[:])

    nc.compile()
    return nc


def _get_nc():
    if "nc" not in _CACHE:
        _CACHE["nc"] = _build_nc()
    return _CACHE["nc"]


def _prep_inputs(x, W, b):
    x = np.asarray(x, dtype=np.float32).astype(NP_BF16)
    # wt[d_lo, k_chunk, o, apk]: W[o,a,p,d,k] -> [d,o,apk] -> [2,128,o,apk] -> [128,2,o,apk]
    wt = np.ascontiguousarray(
        np.asarray(W, dtype=np.float32)
        .astype(NP_BF16)
        .transpose(3, 0, 1, 2, 4)
        .reshape(2, 128, N_OBJ, APK)
        .transpose(1, 0, 2, 3)
    )
    # bt[o2*64+apk, pair] — fp32
    bt = np.ascontiguousarray(
        np.asarray(b, dtype=np.float32)
        .reshape(N_OBJ // 2, 2, APK)
        .transpose(1, 2, 0)
        .reshape(128, N_OBJ // 2)
    )
    in_maps = []
    for c in range(N_CORES):
        xs = x[c * BS : (c + 1) * BS]  # [BS, 16, 256] bf16
        # xt[o, p, k, b] with d = k*128 + p (8KiB contiguous per (o, p))
        xt = np.ascontiguousarray(
            xs.transpose(1, 2, 0).reshape(N_OBJ, 2, 128, BS).transpose(0, 2, 1, 3)
        )
        in_maps.append({"xt": xt, "wt": wt, "bt": bt})
    return in_maps


def kernel(x, W, b, _trace=False, **run_kwargs):
    nc = _get_nc()
    in_maps = _prep_inputs(x, W, b)
    res = run_bass_kernel_spmd(
        nc, in_maps, core_ids=list(range(N_CORES)), trace=_trace, **run_kwargs
    )
    _CACHE["last_results"] = res
    out = np.empty((B, N_OBJ, APK), dtype=np.float32)
    for c in range(N_CORES):
        # out_t[o, apk, batch] -> [batch, o, apk]
        out[c * BS : (c + 1) * BS] = (
            res.results[c]["out"].astype(np.float32).transpose(2, 0, 1)
        )
    return out.reshape(B, N_OBJ, 4, 2, 8)
